# revision 1
# baseline (speedup 1.0000x reference)
"""ConvGeodesic Trainium2 kernel, v2 (signal-formulation).

conv[b,t,m,o] = sum_{q,n} pullback[b,m,t,q,n] * wsum[q,n,o]
pullback[b,m,t,q,n] = sum_c w[b,m,t,q,c] * signal[b, idx[b,m,t,q,c], n]
Then relu, L2-norm argmax over t, pick winning rotation, + bias.

Sharding: m split over 8 cores (3750 rows each), fully local.
Per (b, 128-row block): 15 dma_gathers of 1024 signal rows each (f32,
256B rows; HW caps num_idxs at 1024), weighted c-sum on Pool+DVE, PE
transposes the pullback and contracts (q,n) against wsum into PSUM,
epilogue (relu, norms, angular argmax-pool, bias) on ACT/DVE.
"""

import numpy as np
from contextlib import ExitStack

import concourse.bacc as bacc
import concourse.bass as bass
import concourse.mybir as mybir
import concourse.tile as tile

F32 = mybir.dt.float32
I16 = mybir.dt.int16

B = 2
M = 30000
NCORES = 8
MC = M // NCORES          # 3750 rows per core
T = 8
Q = 5
C3 = 3
NO = 64
TQ = T * Q                # 40
TQC = T * Q * C3          # 120
QN = Q * NO               # 320
QNP = 384                 # padded to 3*128
NIDX = 128 * TQC          # 15360 indices per gather
NW16 = NIDX // 16         # 960


def _cdiv(a, b):
    return (a + b - 1) // b


NG = _cdiv(MC, 128)       # 30 blocks per core
MP = NG * 128             # 3840 padded rows


def bcast(ap, n):
    return ap.to_broadcast(list(ap.shape) + [n])


MUL = mybir.AluOpType.mult


def build_program():
    nc = bacc.Bacc("TRN2", target_bir_lowering=False, debug=False)

    sigd = nc.dram_tensor("sig", [B, M, NO], F32, kind="ExternalInput")
    wsd = nc.dram_tensor("wsd", [3, 128, NO], F32, kind="ExternalInput")
    identd = nc.dram_tensor("ident", [128, 128], F32, kind="ExternalInput")
    w15d = nc.dram_tensor("w15d", [B, NG, 128, TQC], F32, kind="ExternalInput")
    idxd = nc.dram_tensor("idxd", [B, NG, 128, NW16], I16, kind="ExternalInput")
    biasd = nc.dram_tensor("biasd", [MP, NO], F32, kind="ExternalInput")
    outp = nc.dram_tensor("outp", [B, MC, NO], F32, kind="ExternalOutput")

    with tile.TileContext(nc) as tc, ExitStack() as ctx:
        cpool = ctx.enter_context(tc.tile_pool(name="const", bufs=1))
        idxp = ctx.enter_context(tc.tile_pool(name="idxp", bufs=3))
        gp = ctx.enter_context(tc.tile_pool(name="gath", bufs=2))
        t0p = ctx.enter_context(tc.tile_pool(name="t0p", bufs=2))
        t1p = ctx.enter_context(tc.tile_pool(name="t1p", bufs=2))
        ppool = ctx.enter_context(tc.tile_pool(name="ppool", bufs=2))
        pttp = ctx.enter_context(tc.tile_pool(name="pttp", bufs=2))
        tpsum = ctx.enter_context(tc.tile_pool(name="tpsum", bufs=4, space="PSUM"))
        cvpsum = ctx.enter_context(tc.tile_pool(name="cvpsum", bufs=2, space="PSUM"))
        ap_ = ctx.enter_context(tc.tile_pool(name="actp", bufs=2))
        sqp = ctx.enter_context(tc.tile_pool(name="sqp", bufs=2))
        nrmp = ctx.enter_context(tc.tile_pool(name="nrmp", bufs=2))
        plp = ctx.enter_context(tc.tile_pool(name="plp", bufs=2))

        identS = cpool.tile([128, 128], F32)
        nc.sync.dma_start(identS[:], identd[:])
        WS = cpool.tile([128, 3, NO], F32)
        nc.sync.dma_start(WS[:], wsd[:].rearrange("k p n -> p k n"))
        W15 = cpool.tile([128, B, NG, TQC], F32)
        nc.sync.dma_start(W15[:], w15d[:].rearrange("b g p j -> p b g j"))
        biasT = cpool.tile([128, NG, NO], F32)
        nc.sync.dma_start(biasT[:], biasd[:].rearrange("(g p) n -> p g n", p=128))

        for b in range(B):
            for blk in range(NG):
                IB = idxp.tile([128, NW16], I16)
                nc.sync.dma_start(IB[:], idxd[b, blk])

                # HW dma_gather caps at 1024 indices -> 15 sub-gathers
                G = gp.tile([128, TQC, NO], F32)
                for j in range(15):
                    nc.gpsimd.dma_gather(G[:, 8 * j:8 * j + 8, :], sigd[b],
                                         IB[:, 64 * j:64 * j + 64],
                                         1024, 1024, NO)

                # weighted c-sum: P[m, t, q, n] = sum_c w_c * G_c
                # idx order is (c, t, q): each mult depends on 5 sub-gathers
                T0 = t0p.tile([128, TQ, NO], F32)
                T1 = t1p.tile([128, TQ, NO], F32)
                nc.gpsimd.tensor_tensor(T0[:], G[:, 0:TQ, :],
                                        bcast(W15[:, b, blk, 0:TQ], NO), op=MUL)
                nc.gpsimd.tensor_tensor(T1[:], G[:, TQ:2 * TQ, :],
                                        bcast(W15[:, b, blk, TQ:2 * TQ], NO),
                                        op=MUL)
                nc.vector.tensor_add(T0[:], T0[:], T1[:])
                nc.vector.tensor_tensor(T1[:], G[:, 2 * TQ:3 * TQ, :],
                                        bcast(W15[:, b, blk, 2 * TQ:3 * TQ], NO),
                                        op=MUL)
                P = ppool.tile([128, T, QNP], F32)
                nc.scalar.memzero(P[:, :, QN:])
                Pqn = P[:, :, :QN].rearrange("p t (q n) -> p t q n", n=NO)
                nc.vector.tensor_add(
                    Pqn,
                    T0[:].rearrange("p (t q) n -> p t q n", q=Q),
                    T1[:].rearrange("p (t q) n -> p t q n", q=Q))

                # transpose P (24 chunks of 128 cols) -> PTT[f%128, k, m]
                # 4 transposes share one PSUM bank; one grouped copy each
                Pf = P[:].rearrange("p t f -> p (t f)")
                PTT = pttp.tile([128, 24, 128], F32)
                for kb in range(6):
                    tp = tpsum.tile([128, 4, 128], F32, tag="tp")
                    for kk in range(4):
                        k = kb * 4 + kk
                        nc.tensor.transpose(tp[:, kk, :],
                                            Pf[:, k * 128:(k + 1) * 128],
                                            identS[:])
                    nc.scalar.copy(PTT[:, 4 * kb:4 * kb + 4, :], tp[:])

                # conv[m, t, o] accumulated in one PSUM bank
                CV = cvpsum.tile([128, T, NO], F32)
                for t in range(T):
                    for kk in range(3):
                        nc.tensor.matmul(CV[:, t, :], PTT[:, t * 3 + kk, :],
                                         WS[:, kk, :],
                                         start=(kk == 0), stop=(kk == 2))

                # epilogue
                A = ap_.tile([128, T, NO], F32)
                nc.scalar.activation(A[:], CV[:], mybir.ActivationFunctionType.Relu)
                SQ = sqp.tile([128, T, NO], F32)
                nc.scalar.activation(SQ[:], A[:],
                                     mybir.ActivationFunctionType.Square)
                nrm = nrmp.tile([128, 1, T], F32, tag="nrm")
                nc.vector.tensor_reduce(nrm[:, 0, :], SQ[:],
                                        axis=mybir.AxisListType.X,
                                        op=mybir.AluOpType.add)
                mx = nrmp.tile([128, 1], F32, tag="mx")
                nc.vector.tensor_reduce(mx[:], nrm[:], axis=mybir.AxisListType.X,
                                        op=mybir.AluOpType.max)
                msk = nrmp.tile([128, 1, T], F32, tag="msk")
                nc.vector.tensor_tensor(msk[:], nrm[:], bcast(mx[:], T),
                                        op=mybir.AluOpType.is_equal)
                M2 = sqp.tile([128, T, NO], F32, tag="m2")
                nc.vector.tensor_tensor(M2[:], A[:], bcast(msk[:, 0, :], NO),
                                        op=MUL)
                pooled = plp.tile([128, NO], F32)
                nc.vector.tensor_reduce(pooled[:],
                                        M2[:].rearrange("p t n -> p n t"),
                                        axis=mybir.AxisListType.X,
                                        op=mybir.AluOpType.add)
                nc.vector.tensor_add(pooled[:], pooled[:], biasT[:, blk, :])

                r0 = blk * 128
                nv = min(MC - r0, 128)
                nc.sync.dma_start(outp[b, r0:r0 + nv, :], pooled[:nv, :])
    return nc


_CACHE = {}


def _get_program(key=0):
    if key not in _CACHE:
        nc = build_program()
        nc.compile()
        _CACHE[key] = nc
    return _CACHE[key]


def _make_in_maps(signal, bary, weights, bias, mc=MC, ncores=NCORES):
    signal = np.ascontiguousarray(signal, np.float32)
    wsum = np.asarray(weights, np.float32).sum((0, 1))      # (Q, O, N)
    wsqn = wsum.transpose(0, 2, 1).reshape(QN, NO)           # (q*64+n, o)
    wsd = np.zeros((3, 128, NO), np.float32)
    wsd.reshape(QNP, NO)[:QN] = wsqn
    ident = np.eye(128, dtype=np.float32)

    bary = np.asarray(bary, np.float32)
    widx = np.rint(bary[..., 1:6:2]).astype(np.int64)        # (B, M, T, Q, 3)
    wts = bary[..., 0:6:2].astype(np.float32)                # (B, M, T, Q, 3)

    in_maps = []
    for cid in range(ncores):
        m0 = cid * mc
        iv = np.zeros((B, MP, T, Q, C3), np.int64)
        iv[:, :mc] = widx[:, m0:m0 + mc]
        wv = np.zeros((B, MP, T, Q, C3), np.float32)
        wv[:, :mc] = wts[:, m0:m0 + mc]
        # idx order: i = (c*40 + t*5 + q)*128 + p
        iv = iv.reshape(B, NG, 128, T, Q, C3).transpose(0, 1, 2, 5, 3, 4)
        wv = wv.reshape(B, NG, 128, T, Q, C3).transpose(0, 1, 2, 5, 3, 4)
        flat = (iv.reshape(B, NG, 128, TQC)
                .transpose(0, 1, 3, 2)
                .reshape(B, NG, NIDX).astype(np.int16))
        wrapped = flat.reshape(B, NG, NW16, 16).transpose(0, 1, 3, 2)
        idxh = np.ascontiguousarray(
            np.tile(wrapped, (1, 1, 8, 1)))                  # (B, NG, 128, 960)
        w15h = np.ascontiguousarray(
            wv.reshape(B, NG, 128, TQC))                     # (B, NG, 128, TQC)
        bp = np.zeros((MP, NO), np.float32)
        bp[:mc] = bias[m0:m0 + mc]
        in_maps.append(dict(sig=signal, wsd=wsd, ident=ident,
                            w15d=w15h, idxd=idxh, biasd=bp))
    return in_maps


def kernel(signal, bary, weights, bias):
    from concourse.bass_utils import run_bass_kernel_spmd
    nc = _get_program()
    in_maps = _make_in_maps(np.asarray(signal, np.float32),
                            np.asarray(bary, np.float32),
                            np.asarray(weights, np.float32),
                            np.asarray(bias, np.float32))
    res = run_bass_kernel_spmd(nc, in_maps, core_ids=list(range(NCORES)))
    out = np.concatenate([res.results[c]["outp"] for c in range(NCORES)],
                         axis=1)
    return out.astype(np.float32)



# revision 5
# speedup vs baseline: 3.1025x; 3.1025x over previous
"""ConvGeodesic Trainium2 kernel, v3 (Z-table + 4-queue SWDGE gather).

Math: conv[b,t,m,o] = sum_{q,c} w[b,m,t,q,c] * Z[b, idx[b,m,t,q,c], q, o]
where Z[b,v,q,o] = sum_n signal[b,v,n] * wsum[q,n,o], wsum = weights.sum((0,1)).
Then relu, L2-norm argmax over t, pick winning rotation, + bias.

Sharding: m split over 8 cores (3750 rows each), fully local.

Device pipeline per core:
  Prologue: Z = sigT @ wsum via PE matmuls (k=64, f=320), staged through
    PSUM -> SBUF -> HBM scratch (Internal DRAM tile, dep-tracked).
  Main loop over 60 (b, blk) blocks of 128 vertices:
    15x dma_gather (1024 idx each) from Z[b,:,q,:] rows (256B, stride 1280),
    round-robin over 4 SWDGE queues (Q7 core pairs run concurrently, ~2.3
    ns/idx vs 8.5 single-queue).
    DVE: M = G * w (per-partition bcast), reduce over (q,c) -> conv[m,t,o].
    ACT/DVE epilogue: relu, norms, angular argmax-pool, +bias; DMA out.
No per-block PE work at all (the old kernel burned 2.25ms on transposes).
"""

import numpy as np
from contextlib import ExitStack

import concourse.bacc as bacc
import concourse.bass as bass
import concourse.mybir as mybir
import concourse.tile as tile

F32 = mybir.dt.float32
I16 = mybir.dt.int16

B = 2
M = 30000
NCORES = 8
MC = M // NCORES          # 3750 rows per core
T = 8
Q = 5
C3 = 3
NO = 64
NN = 64
SLOTS = Q * C3 * T        # 120 slots per vertex, order (q, c, t)
QO = Q * NO               # 320
NIDX = 1024               # HW cap per dma_gather
NW16 = 128 * SLOTS // 16  # 960 idx free dim per block
NQUEUES = 4


def _cdiv(a, b):
    return (a + b - 1) // b


NG = _cdiv(MC, 128)       # 30 blocks per core
MP = NG * 128             # 3840 padded rows
VP = _cdiv(M, 128) * 128  # 30080 padded table rows
ZCH = 8                   # v-chunks of 128 staged per Z write
SGW = 1920                # signal columns per staging tile


def bcast(ap, n):
    return ap.to_broadcast(list(ap.shape) + [n])


def build_program():
    nc = bacc.Bacc("TRN2", target_bir_lowering=False, debug=False,
                   num_swdge_queues=NQUEUES)

    sigtd = nc.dram_tensor("sigt", [B, NN, VP], F32, kind="ExternalInput")
    wsd = nc.dram_tensor("wsd", [NN, QO], F32, kind="ExternalInput")
    idxd = nc.dram_tensor("idxd", [B, NG, 128, NW16], I16, kind="ExternalInput")
    w15d = nc.dram_tensor("w15d", [B, NG, 128, SLOTS], F32, kind="ExternalInput")
    biasd = nc.dram_tensor("biasd", [MP, NO], F32, kind="ExternalInput")
    outp = nc.dram_tensor("outp", [B, MC, NO], F32, kind="ExternalOutput")

    with tile.TileContext(nc) as tc, ExitStack() as ctx:
        cpool = ctx.enter_context(tc.tile_pool(name="const", bufs=1))
        zdram = ctx.enter_context(tc.tile_pool(name="zdram", bufs=1,
                                               space="DRAM"))
        sgp = ctx.enter_context(tc.tile_pool(name="sgp", bufs=2))
        zpsum = ctx.enter_context(tc.tile_pool(name="zpsum", bufs=4,
                                               space="PSUM"))
        zbp = ctx.enter_context(tc.tile_pool(name="zbp", bufs=2))
        idxp = ctx.enter_context(tc.tile_pool(name="idxp", bufs=2))
        w15p = ctx.enter_context(tc.tile_pool(name="w15p", bufs=2))
        gp = ctx.enter_context(tc.tile_pool(name="gath", bufs=2))
        mp_ = ctx.enter_context(tc.tile_pool(name="mprod", bufs=2))
        cvp = ctx.enter_context(tc.tile_pool(name="cvp", bufs=2))
        ap_ = ctx.enter_context(tc.tile_pool(name="actp", bufs=2))
        sqp = ctx.enter_context(tc.tile_pool(name="sqp", bufs=2))
        nrmp = ctx.enter_context(tc.tile_pool(name="nrmp", bufs=2))
        plp = ctx.enter_context(tc.tile_pool(name="plp", bufs=2))

        WS = cpool.tile([NN, QO], F32)
        nc.sync.dma_start(WS[:], wsd[:])
        biasT = cpool.tile([128, NG, NO], F32)
        nc.sync.dma_start(biasT[:], biasd[:].rearrange("(g p) n -> p g n", p=128))

        # ---- prologue: Z[b, v, q, o] = sigT[b, :, v] @ WS ----
        zts = []
        for b in range(B):
            zt = zdram.tile([VP, Q, NO], F32, tag=f"z{b}", name=f"zt{b}")
            zts.append(zt)
        nvc = VP // 128                     # 235 v-chunks of 128
        for b in range(B):
            for v0 in range(0, nvc, ZCH):   # groups of ZCH chunks
                nch = min(ZCH, nvc - v0)
                zb = zbp.tile([128, ZCH, QO], F32)
                for j in range(nch):
                    vc = v0 + j
                    zp = zpsum.tile([128, QO], F32, tag="zp")
                    sgoff = vc * 128
                    # load signal columns on demand, 1920 at a time
                    nc.tensor.matmul(
                        zp[:],
                        sigtd_sg(nc, sgp, sigtd, b, sgoff),
                        WS[:],
                        start=True, stop=True)
                    nc.scalar.copy(zb[:, j, :], zp[:])
                nc.sync.dma_start(
                    zts[b][v0 * 128:(v0 + nch) * 128, :, :]
                    .rearrange("(j p) q o -> p j (q o)", p=128),
                    zb[:, :nch, :])

        # ---- main loop ----
        gq = 0
        for b in range(B):
            for blk in range(NG):
                IB = idxp.tile([128, NW16], I16)
                nc.sync.dma_start(IB[:], idxd[b, blk])
                W15 = w15p.tile([128, SLOTS], F32)
                nc.sync.dma_start(W15[:], w15d[b, blk])

                G = gp.tile([128, SLOTS, NO], F32)
                for q in range(Q):
                    zsrc = zts[b][:, q, :]          # [VP, 64], stride 320
                    for j3 in range(3):
                        g = q * 3 + j3
                        nc.gpsimd.dma_gather(
                            G[:, q * 24 + 8 * j3: q * 24 + 8 * j3 + 8, :],
                            zsrc, IB[:, 64 * g: 64 * g + 64],
                            NIDX, NIDX, NO, elem_step=Q * NO,
                            queue_num=gq % NQUEUES)
                        gq += 1

                # weighted sum over (q, c): conv[m, t, o]
                MT = mp_.tile([128, SLOTS, NO], F32)
                nc.vector.tensor_tensor(MT[:], G[:], bcast(W15[:], NO),
                                        op=mybir.AluOpType.mult)
                P2 = cvp.tile([128, Q, T * NO], F32, tag="p2")
                nc.vector.tensor_reduce(
                    P2[:],
                    MT[:].rearrange("p (q c t) o -> p q (t o) c", q=Q, c=C3),
                    axis=mybir.AxisListType.X,
                    op=mybir.AluOpType.add)
                CV = cvp.tile([128, T, NO], F32)
                nc.vector.tensor_reduce(
                    CV[:].rearrange("p t o -> p (t o)"),
                    P2[:].rearrange("p q to -> p to q"),
                    axis=mybir.AxisListType.X,
                    op=mybir.AluOpType.add)

                # epilogue
                A = ap_.tile([128, T, NO], F32)
                nc.scalar.activation(A[:], CV[:],
                                     mybir.ActivationFunctionType.Relu)
                SQ = sqp.tile([128, T, NO], F32)
                nc.scalar.activation(SQ[:], A[:],
                                     mybir.ActivationFunctionType.Square)
                nrm = nrmp.tile([128, 1, T], F32, tag="nrm")
                nc.vector.tensor_reduce(nrm[:, 0, :], SQ[:],
                                        axis=mybir.AxisListType.X,
                                        op=mybir.AluOpType.add)
                mx = nrmp.tile([128, 1], F32, tag="mx")
                nc.vector.tensor_reduce(mx[:], nrm[:],
                                        axis=mybir.AxisListType.X,
                                        op=mybir.AluOpType.max)
                msk = nrmp.tile([128, 1, T], F32, tag="msk")
                nc.vector.tensor_tensor(msk[:], nrm[:], bcast(mx[:], T),
                                        op=mybir.AluOpType.is_equal)
                M2 = sqp.tile([128, T, NO], F32, tag="m2")
                nc.vector.tensor_tensor(M2[:], A[:], bcast(msk[:, 0, :], NO),
                                        op=mybir.AluOpType.mult)
                pooled = plp.tile([128, NO], F32)
                nc.vector.tensor_reduce(pooled[:],
                                        M2[:].rearrange("p t n -> p n t"),
                                        axis=mybir.AxisListType.X,
                                        op=mybir.AluOpType.add)
                nc.vector.tensor_add(pooled[:], pooled[:], biasT[:, blk, :])

                r0 = blk * 128
                nv = min(MC - r0, 128)
                nc.sync.dma_start(outp[b, r0:r0 + nv, :], pooled[:nv, :])
    return nc


_SG_CACHE = {}


def sigtd_sg(nc, sgp, sigtd, b, sgoff):
    """Return an AP [64, 128] of signal columns sgoff..sgoff+128, loading a
    [64, SGW] staging tile on demand."""
    w0 = (b, sgoff // SGW)
    if w0 not in _SG_CACHE:
        base = w0[1] * SGW
        wdt = min(SGW, VP - base)
        sg = sgp.tile([NN, SGW], F32, tag=f"sg")
        nc.sync.dma_start(sg[:, :wdt], sigtd[b, :, base:base + wdt])
        _SG_CACHE[w0] = (sg, base)
    sg, base = _SG_CACHE[w0]
    off = sgoff - base
    return sg[:, off:off + 128]


_CACHE = {}


def _get_program(key=0):
    if key not in _CACHE:
        _SG_CACHE.clear()
        nc = build_program()
        nc.compile()
        _CACHE[key] = nc
    return _CACHE[key]


def _make_in_maps(signal, bary, weights, bias, mc=MC, ncores=NCORES):
    signal = np.asarray(signal, np.float32)
    wsum = np.asarray(weights, np.float32).sum((0, 1))      # (Q, O, N)
    wsd = np.ascontiguousarray(
        wsum.transpose(2, 0, 1).reshape(NN, QO))            # [n, (q, o)]

    sigt = np.zeros((B, NN, VP), np.float32)
    sigt[:, :, :M] = signal.transpose(0, 2, 1)              # [b, n, v]

    bary = np.asarray(bary, np.float32)
    widx = np.rint(bary[..., 1:6:2]).astype(np.int64)       # (B, M, T, Q, 3)
    wts = bary[..., 0:6:2].astype(np.float32)               # (B, M, T, Q, 3)

    in_maps = []
    for cid in range(ncores):
        m0 = cid * mc
        iv = np.zeros((B, MP, T, Q, C3), np.int64)
        iv[:, :mc] = widx[:, m0:m0 + mc]
        wv = np.zeros((B, MP, T, Q, C3), np.float32)
        wv[:, :mc] = wts[:, m0:m0 + mc]
        # slot order (q, c, t): i = ((q*3 + c)*8 + t)*128 + m
        iv = iv.reshape(B, NG, 128, T, Q, C3).transpose(0, 1, 2, 4, 5, 3)
        wv = wv.reshape(B, NG, 128, T, Q, C3).transpose(0, 1, 2, 4, 5, 3)
        flat = (iv.reshape(B, NG, 128, SLOTS)
                .transpose(0, 1, 3, 2)
                .reshape(B, NG, 128 * SLOTS).astype(np.int16))
        wrapped = flat.reshape(B, NG, NW16, 16).transpose(0, 1, 3, 2)
        idxh = np.ascontiguousarray(
            np.tile(wrapped, (1, 1, 8, 1)))                  # (B, NG, 128, 960)
        w15h = np.ascontiguousarray(
            wv.reshape(B, NG, 128, SLOTS))                   # (B, NG, 128, 120)
        bp = np.zeros((MP, NO), np.float32)
        bp[:mc] = bias[m0:m0 + mc]
        in_maps.append(dict(sigt=sigt, wsd=wsd, idxd=idxh,
                            w15d=w15h, biasd=bp))
    return in_maps


def kernel(signal, bary, weights, bias):
    from concourse.bass_utils import run_bass_kernel_spmd
    nc = _get_program()
    in_maps = _make_in_maps(np.asarray(signal, np.float32),
                            np.asarray(bary, np.float32),
                            np.asarray(weights, np.float32),
                            np.asarray(bias, np.float32))
    res = run_bass_kernel_spmd(nc, in_maps, core_ids=list(range(NCORES)))
    out = np.concatenate([res.results[c]["outp"] for c in range(NCORES)],
                         axis=1)
    return out.astype(np.float32)


# revision 13
# speedup vs baseline: 3.2835x; 1.0583x over previous
"""ConvGeodesic Trainium2 kernel, v3 (Z-table + 4-queue SWDGE gather).

Math: conv[b,t,m,o] = sum_{q,c} w[b,m,t,q,c] * Z[b, idx[b,m,t,q,c], q, o]
where Z[b,v,q,o] = sum_n signal[b,v,n] * wsum[q,n,o], wsum = weights.sum((0,1)).
Then relu, L2-norm argmax over t, pick winning rotation, + bias.

Sharding: m split over 8 cores (3750 rows each), fully local.

Device pipeline per core:
  Prologue: Z = sigT @ wsum via PE matmuls (k=64, f=320), staged through
    PSUM -> SBUF -> HBM scratch (Internal DRAM tile, dep-tracked).
  Main loop over 60 (b, blk) blocks of 128 vertices:
    15x dma_gather (1024 idx each) from Z[b,:,q,:] rows (256B, stride 1280),
    round-robin over 4 SWDGE queues (Q7 core pairs run concurrently, ~2.3
    ns/idx vs 8.5 single-queue).
    DVE: M = G * w (per-partition bcast), reduce over (q,c) -> conv[m,t,o].
    ACT/DVE epilogue: relu, norms, angular argmax-pool, +bias; DMA out.
No per-block PE work at all (the old kernel burned 2.25ms on transposes).
"""

import numpy as np
import ml_dtypes
from contextlib import ExitStack

import concourse.bacc as bacc
import concourse.bass as bass
import concourse.mybir as mybir
import concourse.tile as tile

F32 = mybir.dt.float32
BF16 = mybir.dt.bfloat16
I16 = mybir.dt.int16

B = 2
M = 30000
NCORES = 8
MC = M // NCORES          # 3750 rows per core
T = 8
Q = 5
C3 = 3
NO = 64
NN = 64
SLOTS = Q * C3 * T        # 120 slots per vertex, order (q, c, t)
QO = Q * NO               # 320
NIDX = 1024               # HW cap per dma_gather
NW16 = 128 * SLOTS // 16  # 960 idx free dim per block
NQUEUES = 4


def _cdiv(a, b):
    return (a + b - 1) // b


NG = _cdiv(MC, 128)       # 30 blocks per core
MP = NG * 128             # 3840 padded rows
VP = _cdiv(M, 128) * 128  # 30080 padded table rows
ZCH = 8                   # v-chunks of 128 staged per Z write
SGW = 1920                # signal columns per staging tile


def bcast(ap, n):
    return ap.to_broadcast(list(ap.shape) + [n])


def build_program():
    nc = bacc.Bacc("TRN2", target_bir_lowering=False, debug=False,
                   num_swdge_queues=NQUEUES)

    sigtd = nc.dram_tensor("sigt", [B, NN, VP], F32, kind="ExternalInput")
    wsd = nc.dram_tensor("wsd", [NN, QO], F32, kind="ExternalInput")
    idxd = nc.dram_tensor("idxd", [B, NG, 128, NW16], I16, kind="ExternalInput")
    w15d = nc.dram_tensor("w15d", [B, NG, 128, SLOTS], F32, kind="ExternalInput")
    biasd = nc.dram_tensor("biasd", [MP, NO], F32, kind="ExternalInput")
    outp = nc.dram_tensor("outp", [B, MC, NO], F32, kind="ExternalOutput")

    with tile.TileContext(nc) as tc, ExitStack() as ctx:
        cpool = ctx.enter_context(tc.tile_pool(name="const", bufs=1))
        zdram = ctx.enter_context(tc.tile_pool(name="zdram", bufs=1,
                                               space="DRAM"))
        sgp = ctx.enter_context(tc.tile_pool(name="sgp", bufs=2))
        zpsum = ctx.enter_context(tc.tile_pool(name="zpsum", bufs=4,
                                               space="PSUM"))
        zbp = ctx.enter_context(tc.tile_pool(name="zbp", bufs=2))
        idxp = ctx.enter_context(tc.tile_pool(name="idxp", bufs=2))
        w15p = ctx.enter_context(tc.tile_pool(name="w15p", bufs=2))
        gp = ctx.enter_context(tc.tile_pool(name="gath", bufs=2))
        mp_ = ctx.enter_context(tc.tile_pool(name="mprod", bufs=2))
        cvp = ctx.enter_context(tc.tile_pool(name="cvp", bufs=2))
        ap_ = ctx.enter_context(tc.tile_pool(name="actp", bufs=2))
        sqp = ctx.enter_context(tc.tile_pool(name="sqp", bufs=2))
        nrmp = ctx.enter_context(tc.tile_pool(name="nrmp", bufs=2))
        plp = ctx.enter_context(tc.tile_pool(name="plp", bufs=2))

        WS = cpool.tile([NN, QO], F32)
        nc.scalar.dma_start(WS[:], wsd[:])
        biasT = cpool.tile([128, NG, NO], F32)
        nc.sync.dma_start(biasT[:], biasd[:].rearrange("(g p) n -> p g n", p=128))

        # ---- prologue: Z[b, v, q, o] = sigT[b, :, v] @ WS ----
        zts = []
        for b in range(B):
            zt = zdram.tile([VP, Q, NO], F32, tag=f"z{b}", name=f"zt{b}")
            zts.append(zt)
        nvc = VP // 128                     # 235 v-chunks of 128
        for b in range(B):
            for v0 in range(0, nvc, ZCH):   # groups of ZCH chunks
                nch = min(ZCH, nvc - v0)
                zb = zbp.tile([128, ZCH, QO], F32)
                for j in range(nch):
                    vc = v0 + j
                    zp = zpsum.tile([128, QO], F32, tag="zp")
                    sgoff = vc * 128
                    # load signal columns on demand, 1920 at a time
                    nc.tensor.matmul(
                        zp[:],
                        sigtd_sg(nc, sgp, sigtd, b, sgoff),
                        WS[:],
                        start=True, stop=True)
                    nc.scalar.copy(zb[:, j, :], zp[:])
                nc.scalar.dma_start(
                    zts[b][v0 * 128:(v0 + nch) * 128, :, :]
                    .rearrange("(j p) q o -> p j (q o)", p=128),
                    zb[:, :nch, :])

        # ---- main loop ----
        gq = 0
        for b in range(B):
            for blk in range(NG):
                IB = idxp.tile([128, NW16], I16)
                nc.sync.dma_start(IB[:], idxd[b, blk])
                W15 = w15p.tile([128, SLOTS], F32)
                nc.sync.dma_start(W15[:], w15d[b, blk])

                G = gp.tile([128, SLOTS, NO], F32)
                for q in range(Q):
                    zsrc = zts[b][:, q, :]          # [VP, 64], stride 320
                    for j3 in range(3):
                        g = q * 3 + j3
                        nc.gpsimd.dma_gather(
                            G[:, q * 24 + 8 * j3: q * 24 + 8 * j3 + 8, :],
                            zsrc, IB[:, 64 * g: 64 * g + 64],
                            NIDX, NIDX, NO, elem_step=Q * NO,
                            queue_num=gq % NQUEUES)
                        gq += 1

                # weighted sum over (q, c): conv[m, t, o]
                MT = mp_.tile([128, SLOTS, NO], F32)
                nc.vector.tensor_tensor(MT[:], G[:], bcast(W15[:], NO),
                                        op=mybir.AluOpType.mult)
                P2 = cvp.tile([128, Q, T * NO], F32, tag="p2")
                nc.vector.tensor_reduce(
                    P2[:],
                    MT[:].rearrange("p (q c t) o -> p q (t o) c", q=Q, c=C3),
                    axis=mybir.AxisListType.X,
                    op=mybir.AluOpType.add)
                CV = cvp.tile([128, T, NO], F32)
                nc.vector.tensor_reduce(
                    CV[:].rearrange("p t o -> p (t o)"),
                    P2[:].rearrange("p q to -> p to q"),
                    axis=mybir.AxisListType.X,
                    op=mybir.AluOpType.add)

                # epilogue
                A = ap_.tile([128, T, NO], F32)
                nc.scalar.activation(A[:], CV[:],
                                     mybir.ActivationFunctionType.Relu)
                SQ = sqp.tile([128, T, NO], F32)
                nc.scalar.activation(SQ[:], A[:],
                                     mybir.ActivationFunctionType.Square)
                nrm = nrmp.tile([128, 1, T], F32, tag="nrm")
                nc.vector.tensor_reduce(nrm[:, 0, :], SQ[:],
                                        axis=mybir.AxisListType.X,
                                        op=mybir.AluOpType.add)
                mx = nrmp.tile([128, 1], F32, tag="mx")
                nc.vector.tensor_reduce(mx[:], nrm[:],
                                        axis=mybir.AxisListType.X,
                                        op=mybir.AluOpType.max)
                msk = nrmp.tile([128, 1, T], F32, tag="msk")
                nc.vector.tensor_tensor(msk[:], nrm[:], bcast(mx[:], T),
                                        op=mybir.AluOpType.is_equal)
                M2 = sqp.tile([128, T, NO], F32, tag="m2")
                nc.vector.tensor_tensor(M2[:], A[:], bcast(msk[:, 0, :], NO),
                                        op=mybir.AluOpType.mult)
                pooled = plp.tile([128, NO], F32)
                nc.vector.tensor_reduce(pooled[:],
                                        M2[:].rearrange("p t n -> p n t"),
                                        axis=mybir.AxisListType.X,
                                        op=mybir.AluOpType.add)
                nc.vector.tensor_add(pooled[:], pooled[:], biasT[:, blk, :])

                r0 = blk * 128
                nv = min(MC - r0, 128)
                nc.sync.dma_start(outp[b, r0:r0 + nv, :], pooled[:nv, :])
    return nc


_SG_CACHE = {}


def sigtd_sg(nc, sgp, sigtd, b, sgoff):
    """Return an AP [64, 128] of signal columns sgoff..sgoff+128, loading a
    [64, SGW] staging tile on demand."""
    w0 = (b, sgoff // SGW)
    if w0 not in _SG_CACHE:
        base = w0[1] * SGW
        wdt = min(SGW, VP - base)
        sg = sgp.tile([NN, SGW], F32, tag="sg")
        nc.scalar.dma_start(sg[:, :wdt], sigtd[b, :, base:base + wdt])
        _SG_CACHE[w0] = (sg, base)
    sg, base = _SG_CACHE[w0]
    off = sgoff - base
    return sg[:, off:off + 128]


_CACHE = {}


def _get_program(key=0):
    if key not in _CACHE:
        _SG_CACHE.clear()
        nc = build_program()
        nc.compile()
        _CACHE[key] = nc
    return _CACHE[key]


def _make_in_maps(signal, bary, weights, bias, mc=MC, ncores=NCORES):
    signal = np.asarray(signal, np.float32)
    wsum = np.asarray(weights, np.float32).sum((0, 1))      # (Q, O, N)
    wsd = np.ascontiguousarray(
        wsum.transpose(2, 0, 1).reshape(NN, QO))

    sigt = np.zeros((B, NN, VP), np.float32)
    sigt[:, :, :M] = signal.transpose(0, 2, 1)              # [b, n, v]

    bary = np.asarray(bary, np.float32)
    widx = np.rint(bary[..., 1:6:2]).astype(np.int64)       # (B, M, T, Q, 3)
    wts = bary[..., 0:6:2].astype(np.float32)               # (B, M, T, Q, 3)

    in_maps = []
    for cid in range(ncores):
        m0 = cid * mc
        iv = np.zeros((B, MP, T, Q, C3), np.int64)
        iv[:, :mc] = widx[:, m0:m0 + mc]
        wv = np.zeros((B, MP, T, Q, C3), np.float32)
        wv[:, :mc] = wts[:, m0:m0 + mc]
        # slot order (q, c, t): i = ((q*3 + c)*8 + t)*128 + m
        iv = iv.reshape(B, NG, 128, T, Q, C3).transpose(0, 1, 2, 4, 5, 3)
        wv = wv.reshape(B, NG, 128, T, Q, C3).transpose(0, 1, 2, 4, 5, 3)
        flat = (iv.reshape(B, NG, 128, SLOTS)
                .transpose(0, 1, 3, 2)
                .reshape(B, NG, 128 * SLOTS).astype(np.int16))
        wrapped = flat.reshape(B, NG, NW16, 16).transpose(0, 1, 3, 2)
        idxh = np.ascontiguousarray(
            np.tile(wrapped, (1, 1, 8, 1)))                  # (B, NG, 128, 960)
        w15h = np.ascontiguousarray(
            wv.reshape(B, NG, 128, SLOTS))                   # (B, NG, 128, 120)
        bp = np.zeros((MP, NO), np.float32)
        bp[:mc] = bias[m0:m0 + mc]
        in_maps.append(dict(sigt=sigt, wsd=wsd, idxd=idxh,
                            w15d=w15h, biasd=bp))
    return in_maps


def kernel(signal, bary, weights, bias):
    from concourse.bass_utils import run_bass_kernel_spmd
    nc = _get_program()
    in_maps = _make_in_maps(np.asarray(signal, np.float32),
                            np.asarray(bary, np.float32),
                            np.asarray(weights, np.float32),
                            np.asarray(bias, np.float32))
    res = run_bass_kernel_spmd(nc, in_maps, core_ids=list(range(NCORES)))
    out = np.concatenate([res.results[c]["outp"] for c in range(NCORES)],
                         axis=1)
    return out.astype(np.float32)


# revision 16
# speedup vs baseline: 3.5487x; 1.0808x over previous
"""ConvGeodesic Trainium2 kernel, v3 (Z-table + 4-queue SWDGE gather).

Math: conv[b,t,m,o] = sum_{q,c} w[b,m,t,q,c] * Z[b, idx[b,m,t,q,c], q, o]
where Z[b,v,q,o] = sum_n signal[b,v,n] * wsum[q,n,o], wsum = weights.sum((0,1)).
Then relu, L2-norm argmax over t, pick winning rotation, + bias.

Sharding: m split over 8 cores (3750 rows each), fully local.

Device pipeline per core:
  Prologue: Z = sigT @ wsum via PE matmuls (k=64, f=320), staged through
    PSUM -> SBUF -> HBM scratch (Internal DRAM tile, dep-tracked).
  Main loop over 60 (b, blk) blocks of 128 vertices:
    15x dma_gather (1024 idx each) from Z[b,:,q,:] rows (256B, stride 1280),
    round-robin over 4 SWDGE queues (Q7 core pairs run concurrently, ~2.3
    ns/idx vs 8.5 single-queue).
    DVE: M = G * w (per-partition bcast), reduce over (q,c) -> conv[m,t,o].
    ACT/DVE epilogue: relu, norms, angular argmax-pool, +bias; DMA out.
No per-block PE work at all (the old kernel burned 2.25ms on transposes).
"""

import numpy as np
import ml_dtypes
from contextlib import ExitStack

import concourse.bacc as bacc
import concourse.bass as bass
import concourse.mybir as mybir
import concourse.tile as tile

F32 = mybir.dt.float32
BF16 = mybir.dt.bfloat16
I16 = mybir.dt.int16

B = 2
M = 30000
NCORES = 8
MC = M // NCORES          # 3750 rows per core
T = 8
Q = 5
C3 = 3
NO = 64
NN = 64
SLOTS = Q * C3 * T        # 120 slots per vertex, order (q, c, t)
QO = Q * NO               # 320
NIDX = 1024               # HW cap per dma_gather
NW16 = 128 * SLOTS // 16  # 960 idx free dim per block
NQUEUES = 4


def _cdiv(a, b):
    return (a + b - 1) // b


NG = _cdiv(MC, 128)       # 30 blocks per core
MP = NG * 128             # 3840 padded rows
VP = _cdiv(M, 128) * 128  # 30080 padded table rows
ZCH = 8                   # v-chunks of 128 staged per Z write
SGW = 1920                # signal columns per staging tile


def bcast(ap, n):
    return ap.to_broadcast(list(ap.shape) + [n])


def build_program():
    nc = bacc.Bacc("TRN2", target_bir_lowering=False, debug=False,
                   num_swdge_queues=NQUEUES)

    sigtd = nc.dram_tensor("sigt", [B, NN, VP], F32, kind="ExternalInput")
    wsd = nc.dram_tensor("wsd", [NN, QO], F32, kind="ExternalInput")
    idxd = nc.dram_tensor("idxd", [B, NG, 128, NW16], I16, kind="ExternalInput")
    w15d = nc.dram_tensor("w15d", [B, NG, 128, SLOTS], F32, kind="ExternalInput")
    biasd = nc.dram_tensor("biasd", [MP, NO], F32, kind="ExternalInput")
    outp = nc.dram_tensor("outp", [B, MC, NO], F32, kind="ExternalOutput")

    with tile.TileContext(nc) as tc, ExitStack() as ctx:
        cpool = ctx.enter_context(tc.tile_pool(name="const", bufs=1))
        zdram = ctx.enter_context(tc.tile_pool(name="zdram", bufs=1,
                                               space="DRAM"))
        sgp = ctx.enter_context(tc.tile_pool(name="sgp", bufs=2))
        zpsum = ctx.enter_context(tc.tile_pool(name="zpsum", bufs=4,
                                               space="PSUM"))
        zbp = ctx.enter_context(tc.tile_pool(name="zbp", bufs=2))
        idxp = ctx.enter_context(tc.tile_pool(name="idxp", bufs=2))
        w15p = ctx.enter_context(tc.tile_pool(name="w15p", bufs=2))
        gp = ctx.enter_context(tc.tile_pool(name="gath", bufs=3))
        cvp = ctx.enter_context(tc.tile_pool(name="cvp", bufs=2))
        ap_ = ctx.enter_context(tc.tile_pool(name="actp", bufs=2))
        sqp = ctx.enter_context(tc.tile_pool(name="sqp", bufs=2))
        nrmp = ctx.enter_context(tc.tile_pool(name="nrmp", bufs=2))
        plp = ctx.enter_context(tc.tile_pool(name="plp", bufs=2))

        WS = cpool.tile([NN, QO], F32)
        nc.scalar.dma_start(WS[:], wsd[:])
        biasT = cpool.tile([128, NG, NO], F32)
        nc.sync.dma_start(biasT[:], biasd[:].rearrange("(g p) n -> p g n", p=128))

        # ---- prologue: Z[b, v, q, o] = sigT[b, :, v] @ WS ----
        zts = []
        for b in range(B):
            zt = zdram.tile([VP, Q, NO], F32, tag=f"z{b}", name=f"zt{b}")
            zts.append(zt)
        nvc = VP // 128                     # 235 v-chunks of 128
        for b in range(B):
            for v0 in range(0, nvc, ZCH):   # groups of ZCH chunks
                nch = min(ZCH, nvc - v0)
                zb = zbp.tile([128, ZCH, QO], F32)
                for j in range(nch):
                    vc = v0 + j
                    zp = zpsum.tile([128, QO], F32, tag="zp")
                    sgoff = vc * 128
                    # load signal columns on demand, 1920 at a time
                    nc.tensor.matmul(
                        zp[:],
                        sigtd_sg(nc, sgp, sigtd, b, sgoff),
                        WS[:],
                        start=True, stop=True)
                    nc.vector.tensor_copy(zb[:, j, :], zp[:])
                nc.scalar.dma_start(
                    zts[b][v0 * 128:(v0 + nch) * 128, :, :]
                    .rearrange("(j p) q o -> p j (q o)", p=128),
                    zb[:, :nch, :])

        # ---- main loop ----
        gq = 0
        for b in range(B):
            for blk in range(NG):
                IB = idxp.tile([128, NW16], I16)
                nc.sync.dma_start(IB[:], idxd[b, blk])
                W15 = w15p.tile([128, SLOTS], F32)
                nc.sync.dma_start(W15[:], w15d[b, blk])

                G = gp.tile([128, SLOTS, NO], F32)
                for q in range(Q):
                    zsrc = zts[b][:, q, :]          # [VP, 64], stride 320
                    for j3 in range(3):
                        g = q * 3 + j3
                        nc.gpsimd.dma_gather(
                            G[:, q * 24 + 8 * j3: q * 24 + 8 * j3 + 8, :],
                            zsrc, IB[:, 64 * g: 64 * g + 64],
                            NIDX, NIDX, NO, elem_step=Q * NO,
                            queue_num=gq % NQUEUES)
                        gq += 1

                # weighted sum over (q, c): conv[m, t, o] (in-place on G)
                nc.vector.tensor_tensor(G[:], G[:], bcast(W15[:], NO),
                                        op=mybir.AluOpType.mult)
                P2 = cvp.tile([128, Q, T * NO], F32, tag="p2")
                nc.vector.tensor_reduce(
                    P2[:],
                    G[:].rearrange("p (q c t) o -> p q (t o) c", q=Q, c=C3),
                    axis=mybir.AxisListType.X,
                    op=mybir.AluOpType.add)
                CV = cvp.tile([128, T, NO], F32)
                nc.vector.tensor_reduce(
                    CV[:].rearrange("p t o -> p (t o)"),
                    P2[:].rearrange("p q to -> p to q"),
                    axis=mybir.AxisListType.X,
                    op=mybir.AluOpType.add)

                # epilogue
                A = ap_.tile([128, T, NO], F32)
                nc.scalar.activation(A[:], CV[:],
                                     mybir.ActivationFunctionType.Relu)
                SQ = sqp.tile([128, T, NO], F32)
                nc.scalar.activation(SQ[:], A[:],
                                     mybir.ActivationFunctionType.Square)
                nrm = nrmp.tile([128, 1, T], F32, tag="nrm")
                nc.vector.tensor_reduce(nrm[:, 0, :], SQ[:],
                                        axis=mybir.AxisListType.X,
                                        op=mybir.AluOpType.add)
                mx = nrmp.tile([128, 1], F32, tag="mx")
                nc.vector.tensor_reduce(mx[:], nrm[:],
                                        axis=mybir.AxisListType.X,
                                        op=mybir.AluOpType.max)
                msk = nrmp.tile([128, 1, T], F32, tag="msk")
                nc.vector.tensor_tensor(msk[:], nrm[:], bcast(mx[:], T),
                                        op=mybir.AluOpType.is_equal)
                M2 = sqp.tile([128, T, NO], F32, tag="m2")
                nc.vector.tensor_tensor(M2[:], A[:], bcast(msk[:, 0, :], NO),
                                        op=mybir.AluOpType.mult)
                pooled = plp.tile([128, NO], F32)
                nc.vector.tensor_reduce(pooled[:],
                                        M2[:].rearrange("p t n -> p n t"),
                                        axis=mybir.AxisListType.X,
                                        op=mybir.AluOpType.add)
                nc.vector.tensor_add(pooled[:], pooled[:], biasT[:, blk, :])

                r0 = blk * 128
                nv = min(MC - r0, 128)
                nc.sync.dma_start(outp[b, r0:r0 + nv, :], pooled[:nv, :])
    return nc


_SG_CACHE = {}


def sigtd_sg(nc, sgp, sigtd, b, sgoff):
    """Return an AP [64, 128] of signal columns sgoff..sgoff+128, loading a
    [64, SGW] staging tile on demand."""
    w0 = (b, sgoff // SGW)
    if w0 not in _SG_CACHE:
        base = w0[1] * SGW
        wdt = min(SGW, VP - base)
        sg = sgp.tile([NN, SGW], F32, tag="sg")
        nc.scalar.dma_start(sg[:, :wdt], sigtd[b, :, base:base + wdt])
        _SG_CACHE[w0] = (sg, base)
    sg, base = _SG_CACHE[w0]
    off = sgoff - base
    return sg[:, off:off + 128]


_CACHE = {}


def _get_program(key=0):
    if key not in _CACHE:
        _SG_CACHE.clear()
        nc = build_program()
        nc.compile()
        _CACHE[key] = nc
    return _CACHE[key]


def _make_in_maps(signal, bary, weights, bias, mc=MC, ncores=NCORES):
    signal = np.asarray(signal, np.float32)
    wsum = np.asarray(weights, np.float32).sum((0, 1))      # (Q, O, N)
    wsd = np.ascontiguousarray(
        wsum.transpose(2, 0, 1).reshape(NN, QO))

    sigt = np.zeros((B, NN, VP), np.float32)
    sigt[:, :, :M] = signal.transpose(0, 2, 1)              # [b, n, v]

    bary = np.asarray(bary, np.float32)
    widx = np.rint(bary[..., 1:6:2]).astype(np.int64)       # (B, M, T, Q, 3)
    wts = bary[..., 0:6:2].astype(np.float32)               # (B, M, T, Q, 3)

    in_maps = []
    for cid in range(ncores):
        m0 = cid * mc
        iv = np.zeros((B, MP, T, Q, C3), np.int64)
        iv[:, :mc] = widx[:, m0:m0 + mc]
        wv = np.zeros((B, MP, T, Q, C3), np.float32)
        wv[:, :mc] = wts[:, m0:m0 + mc]
        # slot order (q, c, t): i = ((q*3 + c)*8 + t)*128 + m
        iv = iv.reshape(B, NG, 128, T, Q, C3).transpose(0, 1, 2, 4, 5, 3)
        wv = wv.reshape(B, NG, 128, T, Q, C3).transpose(0, 1, 2, 4, 5, 3)
        flat = (iv.reshape(B, NG, 128, SLOTS)
                .transpose(0, 1, 3, 2)
                .reshape(B, NG, 128 * SLOTS).astype(np.int16))
        wrapped = flat.reshape(B, NG, NW16, 16).transpose(0, 1, 3, 2)
        idxh = np.ascontiguousarray(
            np.tile(wrapped, (1, 1, 8, 1)))                  # (B, NG, 128, 960)
        w15h = np.ascontiguousarray(
            wv.reshape(B, NG, 128, SLOTS))                   # (B, NG, 128, 120)
        bp = np.zeros((MP, NO), np.float32)
        bp[:mc] = bias[m0:m0 + mc]
        in_maps.append(dict(sigt=sigt, wsd=wsd, idxd=idxh,
                            w15d=w15h, biasd=bp))
    return in_maps


def kernel(signal, bary, weights, bias):
    from concourse.bass_utils import run_bass_kernel_spmd
    nc = _get_program()
    in_maps = _make_in_maps(np.asarray(signal, np.float32),
                            np.asarray(bary, np.float32),
                            np.asarray(weights, np.float32),
                            np.asarray(bias, np.float32))
    res = run_bass_kernel_spmd(nc, in_maps, core_ids=list(range(NCORES)))
    out = np.concatenate([res.results[c]["outp"] for c in range(NCORES)],
                         axis=1)
    return out.astype(np.float32)
